# revision 89
# baseline (speedup 1.0000x reference)
"""Batched dynamic crop + bilinear 64x64 resize (ROIAlign-style) on TRN2.

Contract: kernel(img[64,644,640,3] f32, bbox[64,4] i32) -> [64,3,64,64] f32.
Sharding: pure data parallel, 8 images per NeuronCore across 8 cores.

Per-core algorithm (NB=8 images, img pre-converted to bf16 on host):
  1. All constants (iotas, identities, outer-product coefficient rows) are
     built on-chip via InstIota + memset + small TSPs -- no constant DMAs.
     The only DMA before the gathers is the 128B transposed bbox load.
  2. Gather indices come from two tiny PE outer-product matmuls
     (s = iota (x) crop-dims (+ integer row offset folded in), bias = x1 |
     x1+W) plus a short DVE chain (round-to-int trick, scale by W, add
     bias, convert i32). Clamps are dropped: crop dims >= 64 keep s in
     range, and +1-neighbor overreach only ever carries blend weight 0.
  3. Images are gathered two per indirect-DMA instruction (amortizes the
     SWDGE fixed descriptor-gen cost): idx [128, 2] -> r [128, 2*912] bf16
     row-windows of 304px x 3ch, one 1824B descriptor per row. The gather
     input AP is an overlapping-window DRAM view with the index->element
     coefficient patched back to 3 (pixel units).
  4. Per image, everything not needing image data runs ahead of the
     gather: one-hot rows q0/q1 (fp16 TSPs against iota/iota-1 rows), six
     accumul.. separate PE transposes of q0/q1 chunks, and qt = q0^T+q1^T
     fused into the PSUM->SBUF copy (one DVE TT). After the gather:
     vertical blend pa = R^T @ Wv on PE, pa -> bt fp16 (Act), 9
     accumulating fp16 matmuls -> po, po -> bf16 SBUF copy (Pool), DMA out
     (bf16 halves the store traffic; host upcasts).
"""

import numpy as np

NB = 8            # images per core
NCORES = 8
H, W, C = 644, 640, 3
OUT = 64
Y_OFF = 82
WIN = 304         # gathered window width in pixels (>= max crop width 299)
RLEN = WIN * C    # 912 elements per gathered row window
XCHUNKS = [(0, 128), (128, 128), (256, WIN - 256)]   # x-chunk (start, len)
import os as _os

# images PS..PS+2*NPAIR-1 are gathered two-per-instruction via row-pair
# descriptors; the first PS and the rest get one instruction each
NPAIR = int(_os.environ.get("K_NP", "2"))
PS = int(_os.environ.get("K_PS", "0"))
# NOTE: the SWDGE moves a CONTIGUOUS run of (out.size/num_idx) elements per
# index -- multi-run / strided per-index patterns silently degenerate. Row
# pairs therefore use one flat (W+WIN)-px descriptor.
PAIR2D = bool(int(_os.environ.get("K_P2D", "0")))
_OP = _os.environ.get("K_OP", "2222")
OUT_PAIRS = []                                        # output DMA grouping
_b0 = 0
for _c in _OP.replace("_", ""):
    OUT_PAIRS.append((_b0, int(_c)))
    _b0 += int(_c)
BT_MODE = _os.environ.get("K_BT", "act")              # bt for non-tail images
NTAIL = int(_os.environ.get("K_NTAIL", "0"))          # images with 3-way bt
PSQ_BUFS = int(_os.environ.get("K_PSQ", "1"))
PSO_BUFS = int(_os.environ.get("K_PSO", "3"))
OSB_MODE = _os.environ.get("K_OSB", "dve")            # o_sb engine pattern
PO_LAG = int(_os.environ.get("K_LAG", "3"))           # po stagger depth
WV_ENG = _os.environ.get("K_WV", "pool")              # wv build engine
CMAGIC = 12582912.0                                  # 1.5 * 2^23
DBG = bool(int(_os.environ.get("K_DBG", "0")))

_NC_CACHE = {}


def _build_nc(hoist_waits=True, repeat=1):
    from concourse import bass, mybir
    import concourse.tile as tile
    import bass_rust

    f32 = mybir.dt.float32
    bf16 = mybir.dt.bfloat16
    f16 = mybir.dt.float16
    i32 = mybir.dt.int32
    Alu = mybir.AluOpType
    Act = mybir.ActivationFunctionType

    nc = bass.Bass()
    img = nc.declare_dram_parameter("img", [NB, H, W, C], bf16, isOutput=False)
    bbox = nc.declare_dram_parameter("bbox", [NB, 4], i32, isOutput=False)
    outp = nc.declare_dram_parameter("out", [NB, 3, OUT, OUT], bf16, isOutput=True)
    if DBG:
        d_idx = nc.declare_dram_parameter("d_idx", [128, NB], i32, isOutput=True)
        d_s = nc.declare_dram_parameter("d_s", [128, 2 * NB], f32, isOutput=True)
        d_rhs = nc.declare_dram_parameter("d_rhs", [36, NB], f32, isOutput=True)
        d_l1 = nc.declare_dram_parameter("d_l1", [36, 128], f32, isOutput=True)
        d_l2 = nc.declare_dram_parameter("d_l2", [36, 128], f32, isOutput=True)
        d_q = nc.declare_dram_parameter("d_q", [OUT, 2 * WIN], f16, isOutput=True)
        d_r = nc.declare_dram_parameter("d_r", [128, 2 * RLEN], bf16, isOutput=True)
        d_wv = nc.declare_dram_parameter("d_wv", [128, OUT], bf16, isOutput=True)
        d_w = nc.declare_dram_parameter("d_w", [128, 4 * NB], f32, isOutput=True)

    img_flat = img[:].rearrange("b h w c -> (b h w) c")   # [NB*H*W, 3], coef=3
    # Overlapping-window DRAM views: row i = consecutive elements starting at
    # pixel i -> one descriptor per index. img_win covers one 304px row
    # window (912 el); img_win2 covers a row PAIR (y0 and y0+1 are W px
    # apart, so a (W+WIN)-px span holds both windows: top at cols 0:912,
    # bottom at cols 1920:2832, garbage between).
    npix = NB * H * W
    img_win = bass_rust.AP(img_flat.tensor, 0, [[C, npix - WIN + 1], [1, RLEN]])
    if PAIR2D:
        # two 912-el runs per descriptor, W px apart: rows y0 and y0+1
        PRLEN = 2 * RLEN
        img_win2 = bass_rust.AP(img_flat.tensor, 0,
                                [[C, npix - (W + WIN) + 1], [W * C, 2],
                                 [1, RLEN]])
    else:
        # one flat (W+WIN)-px run per descriptor (garbage gap in the middle)
        PRLEN = (W + WIN) * C
        img_win2 = bass_rust.AP(img_flat.tensor, 0,
                                [[C, npix - (W + WIN) + 1], [1, PRLEN]])
    PBOT = RLEN if PAIR2D else W * C   # bottom-row column offset in r tiles

    def gather_rows(out_ap, idx_ap, in_ap):
        ret = nc.gpsimd.indirect_dma_start(
            out=out_ap,
            out_offset=None,
            in_=in_ap,
            in_offset=bass.IndirectOffsetOnAxis(ap=idx_ap, axis=0),
        )
        # indirect_dma_start derives coef (index->element scale) from the
        # in_ AP's trailing-dim product (912 here); indices are in pixels,
        # so force coef back to 3 elements/pixel.
        inst = ret.ins
        ap0 = inst.ins[0]
        old = ap0.dynamic_ap_info
        ap0.dynamic_ap_info = mybir.DynamicAccessPatternInfo(
            c=old.c,
            actual_ap=old.actual_ap,
            indirect_dim_max_index=old.indirect_dim_max_index,
            offset_expr=[
                mybir.DynamicAccessPatternOffsetExpr(
                    coef=C,
                    aff_expr=mybir.DynamicAccessPatternOffsetExprAffExpr(
                        kind="IndirectArgId", arg_id=1,
                    ),
                )
            ],
        )
        return ret

    with tile.TileContext(nc) as tc:
        with (
            tc.tile_pool(name="const", bufs=1) as cpool,
            tc.tile_pool(name="setup", bufs=1) as spool,
            tc.tile_pool(name="r", bufs=8 - NPAIR) as rpool,
            tc.tile_pool(name="wq", bufs=4) as wqpool,
            tc.tile_pool(name="qt", bufs=4) as qtpool,
            tc.tile_pool(name="bt", bufs=4) as btpool,
            tc.tile_pool(name="osb", bufs=4) as opool,
            tc.tile_pool(name="psQ", bufs=PSQ_BUFS, space="PSUM") as psq,
            tc.tile_pool(name="psA", bufs=int(_os.environ.get("K_PSA", "2")),
                         space="PSUM") as psa,
            tc.tile_pool(name="psO", bufs=PSO_BUFS, space="PSUM") as pso,
        ):
            # ---- the one critical DMA: transposed bbox load ----
            with tc.high_priority():
                rhs_i = spool.tile([4, NB], i32)
                nc.sync.dma_start(out=rhs_i[:], in_=bbox[:].rearrange("b k -> k b"))

            # ---- idx-path constants first ----
            # Engine writes must start at a 32-aligned partition, so the
            # coefficient matrices are built COLUMN-wise in a [128, 2*36]
            # staging tile (writes at partition 0/64 only) and transposed
            # once on PE at setup. rhs layout [36, NB]: row 0 = cb = 82+b*H,
            # rows 32:36 = bbox (x1, y1, x2, y2); rows in between are zero
            # coefficient rows. mm1 -> s (rows 0:64 = s'_y = iota*y2 +
            # (1-iota)*y1 + cb ; rows 64:128 = s_x = iota*(x2-x1)),
            # mm2 -> bias = x1 (the +W for the lower gather half is folded
            # into the idxf TSP instead of a ones-row).
            tc.cur_priority = 30
            KC = 36
            iocol = cpool.tile([OUT, 1], f32)
            nc.gpsimd.iota(iocol[:], [[0, 1]], base=0, channel_multiplier=1,
                           allow_small_or_imprecise_dtypes=True)
            pcol128 = cpool.tile([128, 1], f32)
            nc.gpsimd.iota(pcol128[:], [[0, 1]], base=0, channel_multiplier=1,
                           allow_small_or_imprecise_dtypes=True)
            ixr128 = cpool.tile([128, 128], f32)
            nc.gpsimd.iota(ixr128[:], [[1, 128]], base=0, channel_multiplier=0,
                           allow_small_or_imprecise_dtypes=True)
            eye128 = cpool.tile([128, 128], f32)
            nc.vector.tensor_scalar(eye128[:], ixr128[:], pcol128[:], None,
                                    Alu.is_equal)
            tin = cpool.tile([128, 2 * KC], f32)
            nc.vector.memset(tin[:], 0.0)
            # mm1 coefficient columns (j = (p%64)+0.5 scaled): col 1 = cb coef
            # (1 on the y half), cols 32:36 = bbox coefs
            nc.vector.memset(tin[0:OUT, 0:1], 1.0)
            nc.vector.tensor_scalar(tin[OUT:128, 32:33], iocol[:],
                                    -1.0 / OUT, -1.0 / 128.0, Alu.mult, Alu.add)
            nc.vector.tensor_scalar(tin[0:OUT, 33:34], iocol[:],
                                    -1.0 / OUT, 1.0 - 1.0 / 128.0,
                                    Alu.mult, Alu.add)
            nc.vector.tensor_scalar(tin[OUT:128, 34:35], iocol[:],
                                    1.0 / OUT, 1.0 / 128.0, Alu.mult, Alu.add)
            nc.vector.tensor_scalar(tin[0:OUT, 35:36], iocol[:],
                                    1.0 / OUT, 1.0 / 128.0, Alu.mult, Alu.add)
            # mm2 coefficient columns: col KC+32 (x1) = 1
            nc.vector.memset(tin[:, KC + 32:KC + 33], 1.0)
            lps1 = psq.tile([KC, 128], f32, tag="pq")
            nc.tensor.transpose(out=lps1[:], in_=tin[:, 0:KC],
                                identity=eye128[:])
            lhsT1 = cpool.tile([KC, 128], f32)
            nc.vector.tensor_copy(lhsT1[:], lps1[:])
            lps2 = psq.tile([KC, 128], f32, tag="pq")
            nc.tensor.transpose(out=lps2[:], in_=tin[:, KC:2 * KC],
                                identity=eye128[:])
            lhsT2 = cpool.tile([KC, 128], f32)
            nc.vector.tensor_copy(lhsT2[:], lps2[:])
            rhs = spool.tile([KC, NB], f32)
            nc.gpsimd.iota(rhs[0:1, :], [[H, NB]], base=Y_OFF,
                           channel_multiplier=0,
                           allow_small_or_imprecise_dtypes=True)

            # ---- q-path constants (needed ~4us in) ----
            tc.cur_priority = 60
            pcol = cpool.tile([OUT, 1], f32)
            nc.gpsimd.iota(pcol[:], [[0, 1]], base=0, channel_multiplier=1,
                           allow_small_or_imprecise_dtypes=True)
            ixh = cpool.tile([OUT, WIN], f16)
            nc.gpsimd.iota(ixh[:], [[1, WIN]], base=0, channel_multiplier=0,
                           allow_small_or_imprecise_dtypes=True)

            # transpose identity (f16) and stacked identity (bf16)
            eye16 = cpool.tile([OUT, OUT], f16)
            nc.gpsimd.tensor_scalar(eye16[:], ixh[:, 0:OUT], pcol[:], None,
                                    Alu.is_equal)
            iv = cpool.tile([128, OUT], bf16)
            nc.gpsimd.tensor_scalar(iv[0:OUT, :], ixh[:, 0:OUT], pcol[:], None,
                                    Alu.is_equal)
            nc.gpsimd.tensor_scalar(iv[OUT:128, :], ixh[:, 0:OUT], pcol[:], None,
                                    Alu.is_equal)

            def one_pass():
                # ---- idx-critical chain ----
                with tc.high_priority():
                    nc.vector.tensor_copy(rhs[32:36, :], rhs_i[:])  # i32 -> f32
                    smm = psq.tile([128, 2 * NB], f32, tag="pq")
                    s2 = smm[:, 0:NB]
                    bias3 = smm[:, NB:2 * NB]
                    nc.tensor.matmul(out=s2, lhsT=lhsT1[:], rhs=rhs[:],
                                     start=True, stop=True)
                    nc.tensor.matmul(out=bias3, lhsT=lhsT2[:], rhs=rhs[:],
                                     start=True, stop=True)
                    # t = rne(s - 1) + CMAGIC  (round via f32 magic-number);
                    # one op covers the y half (idx) and x half (weights)
                    t = spool.tile([128, NB], f32, tag="t")
                    nc.vector.tensor_scalar(t[:], s2, 1.0, CMAGIC,
                                            Alu.subtract, Alu.add)
                    idxf = spool.tile([128, NB], f32, tag="idxf")
                    nc.vector.tensor_scalar(idxf[0:OUT, :], t[0:OUT, :], CMAGIC,
                                            float(W), Alu.subtract, Alu.mult)
                    # bottom gather half: (t - C + 1)*W = top + W (row y0+1)
                    nc.vector.tensor_scalar(idxf[OUT:128, :], t[0:OUT, :],
                                            CMAGIC - 1.0, float(W),
                                            Alu.subtract, Alu.mult)
                    # paired images: [128, 1] index columns holding image
                    # 2g's 64 top-row indices (partitions 0:64) then image
                    # 2g+1's (partitions 64:128); the row-pair descriptor
                    # fetches rows y0 and y0+1 as two runs of one span
                    r_tiles = [None] * NB
                    singles = [b for b in range(NB)
                               if b < PS or b >= PS + 2 * NPAIR]
                    if singles:
                        idx_t = spool.tile([128, NB], i32, tag="idx")
                    if PS:
                        nc.vector.scalar_tensor_tensor(
                            out=idx_t[:, 0:PS], in0=idxf[:, 0:PS],
                            scalar=0.0, in1=bias3[:, 0:PS],
                            op0=Alu.add, op1=Alu.add)
                    # leading singles gather first (their images compute
                    # first)
                    for b in range(PS):
                        rs = rpool.tile([128, RLEN], bf16, name=f"rs{b}",
                                        tag="rs")
                        gather_rows(rs[:], idx_t[:, b:b + 1], img_win)
                        r_tiles[b] = ("s", rs, 0)
                    if NPAIR:
                        idxp = spool.tile([128, NPAIR], i32, tag="idxp")
                        nc.vector.scalar_tensor_tensor(
                            out=idxp[0:OUT, :],
                            in0=idxf[0:OUT, PS:PS + 2 * NPAIR:2],
                            scalar=0.0, in1=bias3[0:OUT, PS:PS + 2 * NPAIR:2],
                            op0=Alu.add, op1=Alu.add)
                        nc.vector.scalar_tensor_tensor(
                            out=idxp[OUT:128, :],
                            in0=idxf[0:OUT, PS + 1:PS + 2 * NPAIR:2],
                            scalar=0.0,
                            in1=bias3[0:OUT, PS + 1:PS + 2 * NPAIR:2],
                            op0=Alu.add, op1=Alu.add)
                        for g in range(NPAIR):
                            rp = rpool.tile([128, PRLEN], bf16, name=f"rp{g}",
                                            tag="rp")
                            gather_rows(rp[:], idxp[:, g:g + 1], img_win2)
                            r_tiles[PS + 2 * g] = ("p", rp, 0)
                            r_tiles[PS + 2 * g + 1] = ("p", rp, 64)
                    if PS + 2 * NPAIR < NB:
                        nc.vector.scalar_tensor_tensor(
                            out=idx_t[:, PS + 2 * NPAIR:],
                            in0=idxf[:, PS + 2 * NPAIR:],
                            scalar=0.0, in1=bias3[:, PS + 2 * NPAIR:],
                            op0=Alu.add, op1=Alu.add)

                # trailing single gathers issue just-in-time between the
                # per-image wv builds on the Pool queue
                for b in range(PS + 2 * NPAIR, NB):
                    tc.cur_priority = 2000 + 40 * b - 10
                    rs = rpool.tile([128, RLEN], bf16, name=f"rs{b}",
                                    tag="rs")
                    gather_rows(rs[:], idx_t[:, b:b + 1], img_win)
                    r_tiles[b] = ("s", rs, 0)

                # ---- weight vectors (off the idx path) ----
                tc.cur_priority = 1000
                # x-side: the horizontal blend weight is the hat function
                # max(0, 1-|x - sx|) with sx = s_x - 0.5 (exactly the
                # bilinear weight of columns floor(sx) and floor(sx)+1,
                # ties included) -- no clamps needed since sx is in
                # [0, w-1] for crop widths >= 64
                sxp1 = spool.tile([OUT, NB], f32, tag="sxp1")  # sx + 1
                nc.vector.tensor_scalar(sxp1[:], smm[OUT:128, 0:NB], 0.5, None,
                                        Alu.add)
                sxm1 = spool.tile([OUT, NB], f32, tag="sxm1")  # sx - 1
                nc.vector.tensor_scalar(sxm1[:], smm[OUT:128, 0:NB], 1.5, None,
                                        Alu.subtract)
                # wv2 column stack [1-wy ; wy] for the per-image Wv build
                uy = spool.tile([OUT, NB], f32, tag="uy")
                nc.vector.scalar_tensor_tensor(
                    out=uy[:], in0=t[0:OUT, :], scalar=CMAGIC,
                    in1=smm[0:OUT, 0:NB], op0=Alu.subtract, op1=Alu.subtract)
                wv2 = spool.tile([128, NB], f32, tag="wv2")
                nc.vector.tensor_scalar(wv2[0:OUT, :], uy[:], 1.5, None,
                                        Alu.add)
                nc.vector.tensor_scalar(wv2[OUT:128, :], uy[:], -1.0, -0.5,
                                        Alu.mult, Alu.add)
                if NPAIR:
                    # pair-stacked vertical-weight pointer columns:
                    # col g = [val(img 2g) rows 0:64 ; val(img 2g+1) 64:128]
                    w2p = spool.tile([128, NPAIR], f32, tag="w2p")  # 1-wy
                    wyp = spool.tile([128, NPAIR], f32, tag="wyp")  # wy
                    nc.vector.tensor_scalar(w2p[0:OUT, :],
                                            uy[:, PS:PS + 2 * NPAIR:2],
                                            1.5, None, Alu.add)
                    nc.vector.tensor_scalar(w2p[OUT:128, :],
                                            uy[:, PS + 1:PS + 2 * NPAIR:2],
                                            1.5, None, Alu.add)
                    nc.vector.tensor_scalar(wyp[0:OUT, :],
                                            uy[:, PS:PS + 2 * NPAIR:2],
                                            -1.0, -0.5, Alu.mult, Alu.add)
                    nc.vector.tensor_scalar(wyp[OUT:128, :],
                                            uy[:, PS + 1:PS + 2 * NPAIR:2],
                                            -1.0, -0.5, Alu.mult, Alu.add)

                if DBG:
                    tc.cur_priority = 5000
                    nc.sync.dma_start(out=d_idx[:], in_=idx_t[:])
                    s_sb = spool.tile([128, 2 * NB], f32, tag="dsb")
                    nc.vector.tensor_copy(s_sb[:], smm[:])
                    nc.sync.dma_start(out=d_s[:], in_=s_sb[:])
                    nc.sync.dma_start(out=d_rhs[:], in_=rhs[:])
                    nc.sync.dma_start(out=d_l1[:], in_=lhsT1[:])
                    nc.sync.dma_start(out=d_l2[:], in_=lhsT2[:])
                    nc.sync.dma_start(out=d_w[:, 0:NB], in_=wv2[:])
                    nc.sync.dma_start(out=d_w[0:OUT, NB:2 * NB], in_=sxp1[:])
                    nc.sync.dma_start(out=d_w[0:OUT, 2 * NB:3 * NB], in_=sxm1[:])

                # ---- per-image pipeline ----
                # Priorities stagger the late stages (po/o_sb/out) past the
                # NEXT image's early stages: engines execute in order, so
                # without this, image b's po (waiting on bt) would block
                # image b+1's ready-to-run pa in the PE queue.
                o_tiles = {}
                wvp_tiles = {}
                for b in range(NB):
                    tc.cur_priority = 2000 + 40 * b
                    kind, r_all, rbase = r_tiles[b]

                    # wv build is all-SBUF, so it can live on Pool (the only
                    # engine that may not touch PSUM)
                    wv_eng = nc.gpsimd if WV_ENG == "pool" else nc.vector
                    if kind == "p":
                        g = (b - PS) // 2
                        if g not in wvp_tiles:
                            # [128, 128]: cols 0:64 = stacked diag(1-wy),
                            # cols 64:128 = stacked diag(wy), both images
                            wvp = wqpool.tile([128, 128], bf16, tag="wv",
                                              name=f"wvp{g}")
                            wv_eng.tensor_scalar(wvp[:, 0:OUT], iv[:],
                                                 w2p[:, g:g + 1], None,
                                                 Alu.mult)
                            wv_eng.tensor_scalar(wvp[:, OUT:128], iv[:],
                                                 wyp[:, g:g + 1], None,
                                                 Alu.mult)
                            wvp_tiles[g] = wvp
                        wv = wvp_tiles[g]
                    else:
                        wv = wqpool.tile([128, OUT], bf16, tag="wv")
                        wv_eng.tensor_scalar(wv[:], iv[:], wv2[:, b:b + 1],
                                             None, Alu.mult)

                    # horizontal blend weights: hat function
                    # max(0, 1 - |x - sx_j|) = max(0, min((sx+1) - x,
                    # x - (sx-1))) built in a TSP + an STT; the relu is
                    # fused into the PSUM->SBUF move after the transposes
                    qd = wqpool.tile([OUT, WIN], f16, tag="qd")
                    nc.vector.tensor_scalar(
                        qd[:], ixh[:], -1.0, sxp1[:, b:b + 1],
                        Alu.mult, Alu.add,
                    )
                    qm = wqpool.tile([OUT, WIN], f16, tag="qm")
                    nc.vector.scalar_tensor_tensor(
                        out=qm[:], in0=ixh[:], scalar=sxm1[:, b:b + 1],
                        in1=qd[:], op0=Alu.subtract, op1=Alu.min)
                    if DBG and b == 0:
                        tc.cur_priority = 5000
                        nc.sync.dma_start(out=d_q[:, 0:WIN], in_=qd[:])
                        nc.sync.dma_start(out=d_q[:, WIN:2 * WIN], in_=qm[:])
                        rw = r_all.shape[1] if hasattr(r_all, "shape") else RLEN
                        nc.sync.dma_start(out=d_r[:, 0:rw], in_=r_all[:, 0:rw])
                        nc.sync.dma_start(out=d_wv[:], in_=wv[:, 0:OUT])
                        tc.cur_priority = 2000 + 40 * b + 5
                    pq = psq.tile([128, 3 * OUT], f16, tag="pq")
                    for k, (xs, xn) in enumerate(XCHUNKS):
                        nc.tensor.matmul(
                            out=pq[:xn, k * OUT:(k + 1) * OUT],
                            lhsT=qm[:, xs:xs + xn],
                            rhs=eye16[:],
                            is_transpose=True,
                            start=True,
                            stop=True,
                        )
                    xl = XCHUNKS[-1][1]
                    qt = qtpool.tile([128, 3 * OUT], f16, tag="qt")
                    nc.vector.tensor_scalar(qt[:], pq[:], 0.0, None, Alu.max)

                    tc.cur_priority = 2000 + 40 * b + 10
                    pa = psa.tile([128, 9 * OUT], f32)
                    for k, (xs, xn) in enumerate(XCHUNKS):
                        for c in range(C):
                            osl = pa[:xn, (k * 3 + c) * OUT:(k * 3 + c + 1) * OUT]
                            if kind == "p":
                                # top rows at cols 0:912, bottom rows (y0+1,
                                # W px later) at cols 1920:2832; this image's
                                # 64 row-pairs live at partitions rbase:+64
                                nc.tensor.matmul(
                                    out=osl,
                                    lhsT=r_all[rbase:rbase + OUT,
                                               C * xs + c:C * (xs + xn):C],
                                    rhs=wv[rbase:rbase + OUT, 0:OUT],
                                    start=True,
                                    stop=False,
                                )
                                nc.tensor.matmul(
                                    out=osl,
                                    lhsT=r_all[rbase:rbase + OUT,
                                               PBOT + C * xs + c:
                                               PBOT + C * (xs + xn):C],
                                    rhs=wv[rbase:rbase + OUT, OUT:128],
                                    start=False,
                                    stop=True,
                                )
                            else:
                                nc.tensor.matmul(
                                    out=osl,
                                    lhsT=r_all[:, C * xs + c:C * (xs + xn):C],
                                    rhs=wv[:],
                                    start=True,
                                    stop=True,
                                )
                    tc.cur_priority = 2000 + 40 * b + 30
                    bt = btpool.tile([128, 9 * OUT], f16, tag="bt")
                    if BT_MODE == "tdedic" and b == NB - 1:
                        # last image: full bt on DVE, off the Act conveyor
                        # so the tail runs in parallel
                        nc.vector.tensor_copy(bt[:], pa[:])
                    elif b >= NB - NTAIL:
                        # tail images: split the pa->bt move two ways
                        nc.scalar.activation(bt[:, :6 * OUT], pa[:, :6 * OUT],
                                             Act.Identity)
                        nc.vector.tensor_copy(bt[:xl, 6 * OUT:],
                                              pa[:xl, 6 * OUT:])
                    elif BT_MODE in ("act", "tdedic"):
                        nc.scalar.activation(bt[:], pa[:], Act.Identity)
                    elif BT_MODE == "advd":   # Act main + DVE chunk3
                        nc.scalar.activation(bt[:, :6 * OUT], pa[:, :6 * OUT],
                                             Act.Identity)
                        nc.vector.tensor_copy(bt[:xl, 6 * OUT:],
                                              pa[:xl, 6 * OUT:])
                    else:
                        raise ValueError(BT_MODE)

                    # po matmuls sit past image b+PO_LAG's pa in the PE queue
                    tc.cur_priority = 2000 + 40 * (b + PO_LAG) + 15
                    po = pso.tile([OUT, 3 * OUT], f32)
                    for c in range(C):
                        for k, (xs, xn) in enumerate(XCHUNKS):
                            nc.tensor.matmul(
                                out=po[:, c * OUT:(c + 1) * OUT],
                                lhsT=bt[:xn, (k * 3 + c) * OUT:(k * 3 + c + 1) * OUT],
                                rhs=qt[:xn, k * OUT:(k + 1) * OUT],
                                start=(k == 0),
                                stop=(k == len(XCHUNKS) - 1),
                            )
                    # group images into shared bf16 SBUF tiles
                    tc.cur_priority = 2000 + 40 * (b + PO_LAG) + 25
                    g0, gn = next(p for p in OUT_PAIRS if p[0] <= b < p[0] + p[1])
                    if b == g0:
                        o_tiles[g0] = opool.tile([OUT, gn * 3 * OUT], bf16,
                                                 name=f"osb{g0}", tag="osb")
                    o_sb = o_tiles[g0]
                    sl = o_sb[:, (b - g0) * 3 * OUT:(b - g0 + 1) * 3 * OUT]
                    if OSB_MODE == "split":
                        nc.vector.tensor_copy(sl[:, 0:96], po[:, 0:96])
                        nc.scalar.activation(sl[:, 96:192], po[:, 96:192],
                                             Act.Identity)
                    else:
                        if OSB_MODE == "dve":
                            eng = "dve"
                        elif OSB_MODE == "act":
                            eng = "act"
                        elif OSB_MODE == "altda":
                            eng = ("dve", "act")[b % 2]
                        elif OSB_MODE == "altad":
                            eng = ("act", "dve")[b % 2]
                        else:
                            raise ValueError(OSB_MODE)
                        if eng == "dve":
                            nc.vector.tensor_copy(sl, po[:])
                        else:
                            nc.scalar.activation(sl, po[:], Act.Identity)
                    if b == g0 + gn - 1:
                        tc.cur_priority = 2000 + 40 * (b + PO_LAG) + 30
                        nc.sync.dma_start(
                            out=outp[g0:g0 + gn].rearrange("b c i j -> i b c j"),
                            in_=o_sb[:].rearrange("p (b c j) -> p b c j", b=gn, c=3),
                        )

            for _rep in range(repeat):
                one_pass()

    if hoist_waits:
        _split_excess_waits(nc)
    return nc


def _split_excess_waits(nc):
    """walrus's TT/TS (and possibly other) compute-instruction formats accept
    only one sync-wait command; Tile sometimes attaches two. Hoist extra waits
    onto engine-matched NoOps inserted just before the instruction."""
    from concourse import mybir

    skip = ("InstCall",)
    for fn in nc.m.functions:
        for blk in fn.blocks:
            new_insts = []
            for ins in blk.instructions:
                si = getattr(ins, "sync_info", None)
                waits = list(si.on_wait) if si is not None and si.on_wait else []
                if len(waits) > 1 and type(ins).__name__ not in skip:
                    for i, wt in enumerate(waits[:-1]):
                        nop = mybir.InstNoOp(
                            name=f"{ins.name}_hoistw{i}",
                            ins=[],
                            outs=[],
                        )
                        nop.engine = ins.engine
                        nop.sync_info = mybir.SyncInfo(on_wait=[wt], on_update=[])
                        new_insts.append(nop)
                    si.on_wait = [waits[-1]]
                new_insts.append(ins)
            blk.instructions = new_insts


def _get_nc(repeat=1):
    key = ("nc", repeat)
    if key not in _NC_CACHE:
        _NC_CACHE[key] = _build_nc(repeat=repeat)
    return _NC_CACHE[key]


def _run(img, bbox, **kw):
    import ml_dtypes
    from concourse.bass_utils import run_bass_kernel_spmd

    img = np.ascontiguousarray(
        np.asarray(img, dtype=np.float32).astype(ml_dtypes.bfloat16)
    )
    bbox = np.ascontiguousarray(np.asarray(bbox, dtype=np.int32))
    B = img.shape[0]
    assert B == NB * NCORES and img.shape == (B, H, W, C) and bbox.shape == (B, 4)

    nc = _get_nc()
    in_maps = [
        {"img": img[i * NB: (i + 1) * NB], "bbox": bbox[i * NB: (i + 1) * NB]}
        for i in range(NCORES)
    ]
    return run_bass_kernel_spmd(nc, in_maps, list(range(NCORES)), **kw)


def kernel(img, bbox):
    res = _run(img, bbox)
    return np.concatenate(
        [res.results[i]["out"].astype(np.float32) for i in range(NCORES)], axis=0
    )


# revision 90
# speedup vs baseline: 1.0087x; 1.0087x over previous
"""Batched dynamic crop + bilinear 64x64 resize (ROIAlign-style) on TRN2.

Contract: kernel(img[64,644,640,3] f32, bbox[64,4] i32) -> [64,3,64,64] f32.
Sharding: pure data parallel, 8 images per NeuronCore across 8 cores.

Per-core algorithm (NB=8 images, img pre-converted to bf16 on host):
  1. All constants (iotas, identities, outer-product coefficient rows) are
     built on-chip via InstIota + memset + small TSPs -- no constant DMAs.
     The only DMA before the gathers is the 128B transposed bbox load.
  2. Gather indices come from two tiny PE outer-product matmuls
     (s = iota (x) crop-dims (+ integer row offset folded in), bias = x1 |
     x1+W) plus a short DVE chain (round-to-int trick, scale by W, add
     bias, convert i32). Clamps are dropped: crop dims >= 64 keep s in
     range, and +1-neighbor overreach only ever carries blend weight 0.
  3. Images are gathered two per indirect-DMA instruction (amortizes the
     SWDGE fixed descriptor-gen cost): idx [128, 2] -> r [128, 2*912] bf16
     row-windows of 304px x 3ch, one 1824B descriptor per row. The gather
     input AP is an overlapping-window DRAM view with the index->element
     coefficient patched back to 3 (pixel units).
  4. Per image, everything not needing image data runs ahead of the
     gather: one-hot rows q0/q1 (fp16 TSPs against iota/iota-1 rows), six
     accumul.. separate PE transposes of q0/q1 chunks, and qt = q0^T+q1^T
     fused into the PSUM->SBUF copy (one DVE TT). After the gather:
     vertical blend pa = R^T @ Wv on PE, pa -> bt fp16 (Act), 9
     accumulating fp16 matmuls -> po, po -> bf16 SBUF copy (Pool), DMA out
     (bf16 halves the store traffic; host upcasts).
"""

import numpy as np

NB = 8            # images per core
NCORES = 8
H, W, C = 644, 640, 3
OUT = 64
Y_OFF = 82
WIN = 304         # gathered window width in pixels (>= max crop width 299)
RLEN = WIN * C    # 912 elements per gathered row window
XCHUNKS = [(0, 128), (128, 128), (256, WIN - 256)]   # x-chunk (start, len)
import os as _os

# images PS..PS+2*NPAIR-1 are gathered two-per-instruction via row-pair
# descriptors; the first PS and the rest get one instruction each
NPAIR = int(_os.environ.get("K_NP", "2"))
PS = int(_os.environ.get("K_PS", "0"))
# NOTE: the SWDGE moves a CONTIGUOUS run of (out.size/num_idx) elements per
# index -- multi-run / strided per-index patterns silently degenerate. Row
# pairs therefore use one flat (W+WIN)-px descriptor.
PAIR2D = bool(int(_os.environ.get("K_P2D", "0")))
_OP = _os.environ.get("K_OP", "2222")
OUT_PAIRS = []                                        # output DMA grouping
_b0 = 0
for _c in _OP.replace("_", ""):
    OUT_PAIRS.append((_b0, int(_c)))
    _b0 += int(_c)
BT_MODE = _os.environ.get("K_BT", "act")              # bt for non-tail images
NTAIL = int(_os.environ.get("K_NTAIL", "0"))          # images with 3-way bt
PSQ_BUFS = int(_os.environ.get("K_PSQ", "1"))
PSO_BUFS = int(_os.environ.get("K_PSO", "3"))
OSB_MODE = _os.environ.get("K_OSB", "dve")            # o_sb engine pattern
PO_LAG = int(_os.environ.get("K_LAG", "4"))           # po stagger depth
WV_ENG = _os.environ.get("K_WV", "pool")              # wv build engine
CMAGIC = 12582912.0                                  # 1.5 * 2^23
DBG = bool(int(_os.environ.get("K_DBG", "0")))

_NC_CACHE = {}


def _build_nc(hoist_waits=True, repeat=1):
    from concourse import bass, mybir
    import concourse.tile as tile
    import bass_rust

    f32 = mybir.dt.float32
    bf16 = mybir.dt.bfloat16
    f16 = mybir.dt.float16
    i32 = mybir.dt.int32
    Alu = mybir.AluOpType
    Act = mybir.ActivationFunctionType

    nc = bass.Bass()
    img = nc.declare_dram_parameter("img", [NB, H, W, C], bf16, isOutput=False)
    bbox = nc.declare_dram_parameter("bbox", [NB, 4], i32, isOutput=False)
    outp = nc.declare_dram_parameter("out", [NB, 3, OUT, OUT], bf16, isOutput=True)
    if DBG:
        d_idx = nc.declare_dram_parameter("d_idx", [128, NB], i32, isOutput=True)
        d_s = nc.declare_dram_parameter("d_s", [128, 2 * NB], f32, isOutput=True)
        d_rhs = nc.declare_dram_parameter("d_rhs", [36, NB], f32, isOutput=True)
        d_l1 = nc.declare_dram_parameter("d_l1", [36, 128], f32, isOutput=True)
        d_l2 = nc.declare_dram_parameter("d_l2", [36, 128], f32, isOutput=True)
        d_q = nc.declare_dram_parameter("d_q", [OUT, 2 * WIN], f16, isOutput=True)
        d_r = nc.declare_dram_parameter("d_r", [128, 2 * RLEN], bf16, isOutput=True)
        d_wv = nc.declare_dram_parameter("d_wv", [128, OUT], bf16, isOutput=True)
        d_w = nc.declare_dram_parameter("d_w", [128, 4 * NB], f32, isOutput=True)

    img_flat = img[:].rearrange("b h w c -> (b h w) c")   # [NB*H*W, 3], coef=3
    # Overlapping-window DRAM views: row i = consecutive elements starting at
    # pixel i -> one descriptor per index. img_win covers one 304px row
    # window (912 el); img_win2 covers a row PAIR (y0 and y0+1 are W px
    # apart, so a (W+WIN)-px span holds both windows: top at cols 0:912,
    # bottom at cols 1920:2832, garbage between).
    npix = NB * H * W
    img_win = bass_rust.AP(img_flat.tensor, 0, [[C, npix - WIN + 1], [1, RLEN]])
    if PAIR2D:
        # two 912-el runs per descriptor, W px apart: rows y0 and y0+1
        PRLEN = 2 * RLEN
        img_win2 = bass_rust.AP(img_flat.tensor, 0,
                                [[C, npix - (W + WIN) + 1], [W * C, 2],
                                 [1, RLEN]])
    else:
        # one flat (W+WIN)-px run per descriptor (garbage gap in the middle)
        PRLEN = (W + WIN) * C
        img_win2 = bass_rust.AP(img_flat.tensor, 0,
                                [[C, npix - (W + WIN) + 1], [1, PRLEN]])
    PBOT = RLEN if PAIR2D else W * C   # bottom-row column offset in r tiles

    def gather_rows(out_ap, idx_ap, in_ap):
        ret = nc.gpsimd.indirect_dma_start(
            out=out_ap,
            out_offset=None,
            in_=in_ap,
            in_offset=bass.IndirectOffsetOnAxis(ap=idx_ap, axis=0),
        )
        # indirect_dma_start derives coef (index->element scale) from the
        # in_ AP's trailing-dim product (912 here); indices are in pixels,
        # so force coef back to 3 elements/pixel.
        inst = ret.ins
        ap0 = inst.ins[0]
        old = ap0.dynamic_ap_info
        ap0.dynamic_ap_info = mybir.DynamicAccessPatternInfo(
            c=old.c,
            actual_ap=old.actual_ap,
            indirect_dim_max_index=old.indirect_dim_max_index,
            offset_expr=[
                mybir.DynamicAccessPatternOffsetExpr(
                    coef=C,
                    aff_expr=mybir.DynamicAccessPatternOffsetExprAffExpr(
                        kind="IndirectArgId", arg_id=1,
                    ),
                )
            ],
        )
        return ret

    with tile.TileContext(nc) as tc:
        with (
            tc.tile_pool(name="const", bufs=1) as cpool,
            tc.tile_pool(name="setup", bufs=1) as spool,
            tc.tile_pool(name="r", bufs=8 - NPAIR) as rpool,
            tc.tile_pool(name="wq", bufs=4) as wqpool,
            tc.tile_pool(name="qt", bufs=4) as qtpool,
            tc.tile_pool(name="bt", bufs=4) as btpool,
            tc.tile_pool(name="osb", bufs=4) as opool,
            tc.tile_pool(name="psQ", bufs=PSQ_BUFS, space="PSUM") as psq,
            tc.tile_pool(name="psA", bufs=int(_os.environ.get("K_PSA", "2")),
                         space="PSUM") as psa,
            tc.tile_pool(name="psO", bufs=PSO_BUFS, space="PSUM") as pso,
        ):
            # ---- the one critical DMA: transposed bbox load ----
            with tc.high_priority():
                rhs_i = spool.tile([4, NB], i32)
                nc.sync.dma_start(out=rhs_i[:], in_=bbox[:].rearrange("b k -> k b"))

            # ---- idx-path constants first ----
            # Engine writes must start at a 32-aligned partition, so the
            # coefficient matrices are built COLUMN-wise in a [128, 2*36]
            # staging tile (writes at partition 0/64 only) and transposed
            # once on PE at setup. rhs layout [36, NB]: row 0 = cb = 82+b*H,
            # rows 32:36 = bbox (x1, y1, x2, y2); rows in between are zero
            # coefficient rows. mm1 -> s (rows 0:64 = s'_y = iota*y2 +
            # (1-iota)*y1 + cb ; rows 64:128 = s_x = iota*(x2-x1)),
            # mm2 -> bias = x1 (the +W for the lower gather half is folded
            # into the idxf TSP instead of a ones-row).
            tc.cur_priority = 30
            KC = 36
            iocol = cpool.tile([OUT, 1], f32)
            nc.gpsimd.iota(iocol[:], [[0, 1]], base=0, channel_multiplier=1,
                           allow_small_or_imprecise_dtypes=True)
            pcol128 = cpool.tile([128, 1], f32)
            nc.gpsimd.iota(pcol128[:], [[0, 1]], base=0, channel_multiplier=1,
                           allow_small_or_imprecise_dtypes=True)
            ixr128 = cpool.tile([128, 128], f32)
            nc.gpsimd.iota(ixr128[:], [[1, 128]], base=0, channel_multiplier=0,
                           allow_small_or_imprecise_dtypes=True)
            eye128 = cpool.tile([128, 128], f32)
            nc.vector.tensor_scalar(eye128[:], ixr128[:], pcol128[:], None,
                                    Alu.is_equal)
            tin = cpool.tile([128, 2 * KC], f32)
            nc.vector.memset(tin[:], 0.0)
            # mm1 coefficient columns (j = (p%64)+0.5 scaled): col 1 = cb coef
            # (1 on the y half), cols 32:36 = bbox coefs
            nc.vector.memset(tin[0:OUT, 0:1], 1.0)
            nc.vector.tensor_scalar(tin[OUT:128, 32:33], iocol[:],
                                    -1.0 / OUT, -1.0 / 128.0, Alu.mult, Alu.add)
            nc.vector.tensor_scalar(tin[0:OUT, 33:34], iocol[:],
                                    -1.0 / OUT, 1.0 - 1.0 / 128.0,
                                    Alu.mult, Alu.add)
            nc.vector.tensor_scalar(tin[OUT:128, 34:35], iocol[:],
                                    1.0 / OUT, 1.0 / 128.0, Alu.mult, Alu.add)
            nc.vector.tensor_scalar(tin[0:OUT, 35:36], iocol[:],
                                    1.0 / OUT, 1.0 / 128.0, Alu.mult, Alu.add)
            # mm2 coefficient columns: col KC+32 (x1) = 1
            nc.vector.memset(tin[:, KC + 32:KC + 33], 1.0)
            lps1 = psq.tile([KC, 128], f32, tag="pq")
            nc.tensor.transpose(out=lps1[:], in_=tin[:, 0:KC],
                                identity=eye128[:])
            lhsT1 = cpool.tile([KC, 128], f32)
            nc.vector.tensor_copy(lhsT1[:], lps1[:])
            lps2 = psq.tile([KC, 128], f32, tag="pq")
            nc.tensor.transpose(out=lps2[:], in_=tin[:, KC:2 * KC],
                                identity=eye128[:])
            lhsT2 = cpool.tile([KC, 128], f32)
            nc.vector.tensor_copy(lhsT2[:], lps2[:])
            rhs = spool.tile([KC, NB], f32)
            nc.gpsimd.iota(rhs[0:1, :], [[H, NB]], base=Y_OFF,
                           channel_multiplier=0,
                           allow_small_or_imprecise_dtypes=True)

            # ---- q-path constants (needed ~4us in) ----
            tc.cur_priority = 60
            pcol = cpool.tile([OUT, 1], f32)
            nc.gpsimd.iota(pcol[:], [[0, 1]], base=0, channel_multiplier=1,
                           allow_small_or_imprecise_dtypes=True)
            ixh = cpool.tile([OUT, WIN], f16)
            nc.gpsimd.iota(ixh[:], [[1, WIN]], base=0, channel_multiplier=0,
                           allow_small_or_imprecise_dtypes=True)

            # transpose identity (f16) and stacked identity (bf16)
            eye16 = cpool.tile([OUT, OUT], f16)
            nc.gpsimd.tensor_scalar(eye16[:], ixh[:, 0:OUT], pcol[:], None,
                                    Alu.is_equal)
            iv = cpool.tile([128, OUT], bf16)
            nc.gpsimd.tensor_scalar(iv[0:OUT, :], ixh[:, 0:OUT], pcol[:], None,
                                    Alu.is_equal)
            nc.gpsimd.tensor_scalar(iv[OUT:128, :], ixh[:, 0:OUT], pcol[:], None,
                                    Alu.is_equal)

            def one_pass():
                # ---- idx-critical chain ----
                with tc.high_priority():
                    nc.vector.tensor_copy(rhs[32:36, :], rhs_i[:])  # i32 -> f32
                    smm = psq.tile([128, 2 * NB], f32, tag="pq")
                    s2 = smm[:, 0:NB]
                    bias3 = smm[:, NB:2 * NB]
                    nc.tensor.matmul(out=s2, lhsT=lhsT1[:], rhs=rhs[:],
                                     start=True, stop=True)
                    nc.tensor.matmul(out=bias3, lhsT=lhsT2[:], rhs=rhs[:],
                                     start=True, stop=True)
                    # t = rne(s - 1) + CMAGIC  (round via f32 magic-number);
                    # one op covers the y half (idx) and x half (weights)
                    t = spool.tile([128, NB], f32, tag="t")
                    nc.vector.tensor_scalar(t[:], s2, 1.0, CMAGIC,
                                            Alu.subtract, Alu.add)
                    idxf = spool.tile([128, NB], f32, tag="idxf")
                    nc.vector.tensor_scalar(idxf[0:OUT, :], t[0:OUT, :], CMAGIC,
                                            float(W), Alu.subtract, Alu.mult)
                    # bottom gather half: (t - C + 1)*W = top + W (row y0+1)
                    nc.vector.tensor_scalar(idxf[OUT:128, :], t[0:OUT, :],
                                            CMAGIC - 1.0, float(W),
                                            Alu.subtract, Alu.mult)
                    # paired images: [128, 1] index columns holding image
                    # 2g's 64 top-row indices (partitions 0:64) then image
                    # 2g+1's (partitions 64:128); the row-pair descriptor
                    # fetches rows y0 and y0+1 as two runs of one span
                    r_tiles = [None] * NB
                    singles = [b for b in range(NB)
                               if b < PS or b >= PS + 2 * NPAIR]
                    if singles:
                        idx_t = spool.tile([128, NB], i32, tag="idx")
                    if PS:
                        nc.vector.scalar_tensor_tensor(
                            out=idx_t[:, 0:PS], in0=idxf[:, 0:PS],
                            scalar=0.0, in1=bias3[:, 0:PS],
                            op0=Alu.add, op1=Alu.add)
                    # leading singles gather first (their images compute
                    # first)
                    for b in range(PS):
                        rs = rpool.tile([128, RLEN], bf16, name=f"rs{b}",
                                        tag="rs")
                        gather_rows(rs[:], idx_t[:, b:b + 1], img_win)
                        r_tiles[b] = ("s", rs, 0)
                    if NPAIR:
                        idxp = spool.tile([128, NPAIR], i32, tag="idxp")
                        nc.vector.scalar_tensor_tensor(
                            out=idxp[0:OUT, :],
                            in0=idxf[0:OUT, PS:PS + 2 * NPAIR:2],
                            scalar=0.0, in1=bias3[0:OUT, PS:PS + 2 * NPAIR:2],
                            op0=Alu.add, op1=Alu.add)
                        nc.vector.scalar_tensor_tensor(
                            out=idxp[OUT:128, :],
                            in0=idxf[0:OUT, PS + 1:PS + 2 * NPAIR:2],
                            scalar=0.0,
                            in1=bias3[0:OUT, PS + 1:PS + 2 * NPAIR:2],
                            op0=Alu.add, op1=Alu.add)
                        for g in range(NPAIR):
                            rp = rpool.tile([128, PRLEN], bf16, name=f"rp{g}",
                                            tag="rp")
                            gather_rows(rp[:], idxp[:, g:g + 1], img_win2)
                            r_tiles[PS + 2 * g] = ("p", rp, 0)
                            r_tiles[PS + 2 * g + 1] = ("p", rp, 64)
                    if PS + 2 * NPAIR < NB:
                        nc.vector.scalar_tensor_tensor(
                            out=idx_t[:, PS + 2 * NPAIR:],
                            in0=idxf[:, PS + 2 * NPAIR:],
                            scalar=0.0, in1=bias3[:, PS + 2 * NPAIR:],
                            op0=Alu.add, op1=Alu.add)

                # trailing single gathers issue just-in-time between the
                # per-image wv builds on the Pool queue
                for b in range(PS + 2 * NPAIR, NB):
                    tc.cur_priority = 2000 + 40 * b - 10
                    rs = rpool.tile([128, RLEN], bf16, name=f"rs{b}",
                                    tag="rs")
                    gather_rows(rs[:], idx_t[:, b:b + 1], img_win)
                    r_tiles[b] = ("s", rs, 0)

                # ---- weight vectors (off the idx path) ----
                tc.cur_priority = 1000
                # x-side: the horizontal blend weight is the hat function
                # max(0, 1-|x - sx|) with sx = s_x - 0.5 (exactly the
                # bilinear weight of columns floor(sx) and floor(sx)+1,
                # ties included) -- no clamps needed since sx is in
                # [0, w-1] for crop widths >= 64
                sxp1 = spool.tile([OUT, NB], f32, tag="sxp1")  # sx + 1
                nc.vector.tensor_scalar(sxp1[:], smm[OUT:128, 0:NB], 0.5, None,
                                        Alu.add)
                sxm1 = spool.tile([OUT, NB], f32, tag="sxm1")  # sx - 1
                nc.vector.tensor_scalar(sxm1[:], smm[OUT:128, 0:NB], 1.5, None,
                                        Alu.subtract)
                # wv2 column stack [1-wy ; wy] for the per-image Wv build
                uy = spool.tile([OUT, NB], f32, tag="uy")
                nc.vector.scalar_tensor_tensor(
                    out=uy[:], in0=t[0:OUT, :], scalar=CMAGIC,
                    in1=smm[0:OUT, 0:NB], op0=Alu.subtract, op1=Alu.subtract)
                wv2 = spool.tile([128, NB], f32, tag="wv2")
                nc.vector.tensor_scalar(wv2[0:OUT, :], uy[:], 1.5, None,
                                        Alu.add)
                nc.vector.tensor_scalar(wv2[OUT:128, :], uy[:], -1.0, -0.5,
                                        Alu.mult, Alu.add)
                if NPAIR:
                    # pair-stacked vertical-weight pointer columns:
                    # col g = [val(img 2g) rows 0:64 ; val(img 2g+1) 64:128]
                    w2p = spool.tile([128, NPAIR], f32, tag="w2p")  # 1-wy
                    wyp = spool.tile([128, NPAIR], f32, tag="wyp")  # wy
                    nc.vector.tensor_scalar(w2p[0:OUT, :],
                                            uy[:, PS:PS + 2 * NPAIR:2],
                                            1.5, None, Alu.add)
                    nc.vector.tensor_scalar(w2p[OUT:128, :],
                                            uy[:, PS + 1:PS + 2 * NPAIR:2],
                                            1.5, None, Alu.add)
                    nc.vector.tensor_scalar(wyp[0:OUT, :],
                                            uy[:, PS:PS + 2 * NPAIR:2],
                                            -1.0, -0.5, Alu.mult, Alu.add)
                    nc.vector.tensor_scalar(wyp[OUT:128, :],
                                            uy[:, PS + 1:PS + 2 * NPAIR:2],
                                            -1.0, -0.5, Alu.mult, Alu.add)

                if DBG:
                    tc.cur_priority = 5000
                    nc.sync.dma_start(out=d_idx[:], in_=idx_t[:])
                    s_sb = spool.tile([128, 2 * NB], f32, tag="dsb")
                    nc.vector.tensor_copy(s_sb[:], smm[:])
                    nc.sync.dma_start(out=d_s[:], in_=s_sb[:])
                    nc.sync.dma_start(out=d_rhs[:], in_=rhs[:])
                    nc.sync.dma_start(out=d_l1[:], in_=lhsT1[:])
                    nc.sync.dma_start(out=d_l2[:], in_=lhsT2[:])
                    nc.sync.dma_start(out=d_w[:, 0:NB], in_=wv2[:])
                    nc.sync.dma_start(out=d_w[0:OUT, NB:2 * NB], in_=sxp1[:])
                    nc.sync.dma_start(out=d_w[0:OUT, 2 * NB:3 * NB], in_=sxm1[:])

                # ---- per-image pipeline ----
                # Priorities stagger the late stages (po/o_sb/out) past the
                # NEXT image's early stages: engines execute in order, so
                # without this, image b's po (waiting on bt) would block
                # image b+1's ready-to-run pa in the PE queue.
                o_tiles = {}
                wvp_tiles = {}
                for b in range(NB):
                    tc.cur_priority = 2000 + 40 * b
                    kind, r_all, rbase = r_tiles[b]

                    # wv build is all-SBUF, so it can live on Pool (the only
                    # engine that may not touch PSUM)
                    wv_eng = nc.gpsimd if WV_ENG == "pool" else nc.vector
                    if kind == "p":
                        g = (b - PS) // 2
                        if g not in wvp_tiles:
                            # [128, 128]: cols 0:64 = stacked diag(1-wy),
                            # cols 64:128 = stacked diag(wy), both images
                            wvp = wqpool.tile([128, 128], bf16, tag="wv",
                                              name=f"wvp{g}")
                            wv_eng.tensor_scalar(wvp[:, 0:OUT], iv[:],
                                                 w2p[:, g:g + 1], None,
                                                 Alu.mult)
                            wv_eng.tensor_scalar(wvp[:, OUT:128], iv[:],
                                                 wyp[:, g:g + 1], None,
                                                 Alu.mult)
                            wvp_tiles[g] = wvp
                        wv = wvp_tiles[g]
                    else:
                        wv = wqpool.tile([128, OUT], bf16, tag="wv")
                        wv_eng.tensor_scalar(wv[:], iv[:], wv2[:, b:b + 1],
                                             None, Alu.mult)

                    # horizontal blend weights: hat function
                    # max(0, 1 - |x - sx_j|) = max(0, min((sx+1) - x,
                    # x - (sx-1))) built in a TSP + an STT; the relu is
                    # fused into the PSUM->SBUF move after the transposes
                    qd = wqpool.tile([OUT, WIN], f16, tag="qd")
                    nc.vector.tensor_scalar(
                        qd[:], ixh[:], -1.0, sxp1[:, b:b + 1],
                        Alu.mult, Alu.add,
                    )
                    qm = wqpool.tile([OUT, WIN], f16, tag="qm")
                    nc.vector.scalar_tensor_tensor(
                        out=qm[:], in0=ixh[:], scalar=sxm1[:, b:b + 1],
                        in1=qd[:], op0=Alu.subtract, op1=Alu.min)
                    if DBG and b == 0:
                        tc.cur_priority = 5000
                        nc.sync.dma_start(out=d_q[:, 0:WIN], in_=qd[:])
                        nc.sync.dma_start(out=d_q[:, WIN:2 * WIN], in_=qm[:])
                        rw = r_all.shape[1] if hasattr(r_all, "shape") else RLEN
                        nc.sync.dma_start(out=d_r[:, 0:rw], in_=r_all[:, 0:rw])
                        nc.sync.dma_start(out=d_wv[:], in_=wv[:, 0:OUT])
                        tc.cur_priority = 2000 + 40 * b + 5
                    pq = psq.tile([128, 3 * OUT], f16, tag="pq")
                    for k, (xs, xn) in enumerate(XCHUNKS):
                        nc.tensor.matmul(
                            out=pq[:xn, k * OUT:(k + 1) * OUT],
                            lhsT=qm[:, xs:xs + xn],
                            rhs=eye16[:],
                            is_transpose=True,
                            start=True,
                            stop=True,
                        )
                    xl = XCHUNKS[-1][1]
                    qt = qtpool.tile([128, 3 * OUT], f16, tag="qt")
                    nc.vector.tensor_scalar(qt[:], pq[:], 0.0, None, Alu.max)

                    tc.cur_priority = 2000 + 40 * b + 10
                    pa = psa.tile([128, 9 * OUT], f32)
                    for k, (xs, xn) in enumerate(XCHUNKS):
                        for c in range(C):
                            osl = pa[:xn, (k * 3 + c) * OUT:(k * 3 + c + 1) * OUT]
                            if kind == "p":
                                # top rows at cols 0:912, bottom rows (y0+1,
                                # W px later) at cols 1920:2832; this image's
                                # 64 row-pairs live at partitions rbase:+64
                                nc.tensor.matmul(
                                    out=osl,
                                    lhsT=r_all[rbase:rbase + OUT,
                                               C * xs + c:C * (xs + xn):C],
                                    rhs=wv[rbase:rbase + OUT, 0:OUT],
                                    start=True,
                                    stop=False,
                                )
                                nc.tensor.matmul(
                                    out=osl,
                                    lhsT=r_all[rbase:rbase + OUT,
                                               PBOT + C * xs + c:
                                               PBOT + C * (xs + xn):C],
                                    rhs=wv[rbase:rbase + OUT, OUT:128],
                                    start=False,
                                    stop=True,
                                )
                            else:
                                nc.tensor.matmul(
                                    out=osl,
                                    lhsT=r_all[:, C * xs + c:C * (xs + xn):C],
                                    rhs=wv[:],
                                    start=True,
                                    stop=True,
                                )
                    tc.cur_priority = 2000 + 40 * b + 30
                    bt = btpool.tile([128, 9 * OUT], f16, tag="bt")
                    if BT_MODE == "tdedic" and b == NB - 1:
                        # last image: full bt on DVE, off the Act conveyor
                        # so the tail runs in parallel
                        nc.vector.tensor_copy(bt[:], pa[:])
                    elif b >= NB - NTAIL:
                        # tail images: split the pa->bt move two ways
                        nc.scalar.activation(bt[:, :6 * OUT], pa[:, :6 * OUT],
                                             Act.Identity)
                        nc.vector.tensor_copy(bt[:xl, 6 * OUT:],
                                              pa[:xl, 6 * OUT:])
                    elif BT_MODE in ("act", "tdedic"):
                        nc.scalar.activation(bt[:], pa[:], Act.Identity)
                    elif BT_MODE == "advd":   # Act main + DVE chunk3
                        nc.scalar.activation(bt[:, :6 * OUT], pa[:, :6 * OUT],
                                             Act.Identity)
                        nc.vector.tensor_copy(bt[:xl, 6 * OUT:],
                                              pa[:xl, 6 * OUT:])
                    else:
                        raise ValueError(BT_MODE)

                    # po matmuls sit past image b+PO_LAG's pa in the PE queue
                    tc.cur_priority = 2000 + 40 * (b + PO_LAG) + 15
                    po = pso.tile([OUT, 3 * OUT], f32)
                    for c in range(C):
                        for k, (xs, xn) in enumerate(XCHUNKS):
                            nc.tensor.matmul(
                                out=po[:, c * OUT:(c + 1) * OUT],
                                lhsT=bt[:xn, (k * 3 + c) * OUT:(k * 3 + c + 1) * OUT],
                                rhs=qt[:xn, k * OUT:(k + 1) * OUT],
                                start=(k == 0),
                                stop=(k == len(XCHUNKS) - 1),
                            )
                    # group images into shared bf16 SBUF tiles
                    tc.cur_priority = 2000 + 40 * (b + PO_LAG) + 25
                    g0, gn = next(p for p in OUT_PAIRS if p[0] <= b < p[0] + p[1])
                    if b == g0:
                        o_tiles[g0] = opool.tile([OUT, gn * 3 * OUT], bf16,
                                                 name=f"osb{g0}", tag="osb")
                    o_sb = o_tiles[g0]
                    sl = o_sb[:, (b - g0) * 3 * OUT:(b - g0 + 1) * 3 * OUT]
                    if OSB_MODE == "split":
                        nc.vector.tensor_copy(sl[:, 0:96], po[:, 0:96])
                        nc.scalar.activation(sl[:, 96:192], po[:, 96:192],
                                             Act.Identity)
                    else:
                        if OSB_MODE == "dve":
                            eng = "dve"
                        elif OSB_MODE == "act":
                            eng = "act"
                        elif OSB_MODE == "altda":
                            eng = ("dve", "act")[b % 2]
                        elif OSB_MODE == "altad":
                            eng = ("act", "dve")[b % 2]
                        else:
                            raise ValueError(OSB_MODE)
                        if eng == "dve":
                            nc.vector.tensor_copy(sl, po[:])
                        else:
                            nc.scalar.activation(sl, po[:], Act.Identity)
                    if b == g0 + gn - 1:
                        tc.cur_priority = 2000 + 40 * (b + PO_LAG) + 30
                        nc.sync.dma_start(
                            out=outp[g0:g0 + gn].rearrange("b c i j -> i b c j"),
                            in_=o_sb[:].rearrange("p (b c j) -> p b c j", b=gn, c=3),
                        )

            for _rep in range(repeat):
                one_pass()

    if hoist_waits:
        _split_excess_waits(nc)
    return nc


def _split_excess_waits(nc):
    """walrus's TT/TS (and possibly other) compute-instruction formats accept
    only one sync-wait command; Tile sometimes attaches two. Hoist extra waits
    onto engine-matched NoOps inserted just before the instruction."""
    from concourse import mybir

    skip = ("InstCall",)
    for fn in nc.m.functions:
        for blk in fn.blocks:
            new_insts = []
            for ins in blk.instructions:
                si = getattr(ins, "sync_info", None)
                waits = list(si.on_wait) if si is not None and si.on_wait else []
                if len(waits) > 1 and type(ins).__name__ not in skip:
                    for i, wt in enumerate(waits[:-1]):
                        nop = mybir.InstNoOp(
                            name=f"{ins.name}_hoistw{i}",
                            ins=[],
                            outs=[],
                        )
                        nop.engine = ins.engine
                        nop.sync_info = mybir.SyncInfo(on_wait=[wt], on_update=[])
                        new_insts.append(nop)
                    si.on_wait = [waits[-1]]
                new_insts.append(ins)
            blk.instructions = new_insts


def _get_nc(repeat=1):
    key = ("nc", repeat)
    if key not in _NC_CACHE:
        _NC_CACHE[key] = _build_nc(repeat=repeat)
    return _NC_CACHE[key]


def _run(img, bbox, **kw):
    import ml_dtypes
    from concourse.bass_utils import run_bass_kernel_spmd

    img = np.ascontiguousarray(
        np.asarray(img, dtype=np.float32).astype(ml_dtypes.bfloat16)
    )
    bbox = np.ascontiguousarray(np.asarray(bbox, dtype=np.int32))
    B = img.shape[0]
    assert B == NB * NCORES and img.shape == (B, H, W, C) and bbox.shape == (B, 4)

    nc = _get_nc()
    in_maps = [
        {"img": img[i * NB: (i + 1) * NB], "bbox": bbox[i * NB: (i + 1) * NB]}
        for i in range(NCORES)
    ]
    return run_bass_kernel_spmd(nc, in_maps, list(range(NCORES)), **kw)


def kernel(img, bbox):
    res = _run(img, bbox)
    return np.concatenate(
        [res.results[i]["out"].astype(np.float32) for i in range(NCORES)], axis=0
    )
